# revision 31
# baseline (speedup 1.0000x reference)
"""Trainium2 Bass kernel for nn_Conv2DExperimental (MVN-sampled 3x3 conv).

Computation (per the nn.Module):
  L    = tril(weight_L, -1) + softplus(diag(weight_L)) * I      # [O,I,D,D], D=9
  w    = weight_loc + einsum('oiab,oib->oia', L, eps_w)         # [O,I,3,3]
  b    = bias_loc + eps_b * softplus(bias_ro)                   # [O]
  out  = conv2d(x, w, SAME, NCHW) + b

Distribution: data-parallel over the batch dim of x (32 images -> 8 cores x 4),
with the weight sampling replicated on every core (it is tiny).

Per-core kernel:
  - sampling runs on VectorE/ScalarE with O on the partition dim; the 9 sampled
    64x64 tap matrices are PE-transposed into block-diagonal [128,128] bf16
    lhsT tiles pairing two images per matmul (partitions = (image, channel)).
  - conv runs tap-OUTER over 8 PSUM banks (16 output rows per strip): the 8
    consecutive matmuls of one tap pass share the same lhsT tile, amortizing
    the per-matmul weight-(re)load/dispatch bubble 8x.
  - x and out live in HBM as bf16 (host casts); SBUF strips are packed
    (no halo columns) - SAME padding is realized by restricting the
    out-column/row ranges of edge-tap matmuls, so every strip load/store is
    one contiguous descriptor per partition.
  - PSUM evacuation (bias add fused, bf16 out) alternates ScalarE/VectorE.
"""

import sys
from contextlib import ExitStack

for _p in ("/opt/trn_rl_repo",):
    if _p not in sys.path:
        sys.path.insert(0, _p)

import numpy as np
import ml_dtypes

import concourse.bass as bass
import concourse.bacc as bacc
import concourse.mybir as mybir
from concourse.tile import TileContext

F32 = mybir.dt.float32
F32R = mybir.dt.float32r
BF16 = mybir.dt.bfloat16
AF = mybir.ActivationFunctionType

N_CORES = 8
O = 64
I = 64
KK = 3
D = KK * KK  # 9

# tap order within each PSUM accumulation group: (0,0) first (it is never
# edge-restricted, so the start=True matmul always zeroes the full bank tile)
TAP_ORDER = [4, 0, 1, 2, 3, 5, 6, 7, 8]


def build_nc(nb=4, hh=224, ww=224, rstrip=16, x_bufs=3, o_bufs=2, n_warm=74):
    """Build the per-core Bass program.

    nb: images per core (must be even: images are processed in pairs)
    hh, ww: spatial dims; rstrip: output rows per strip (8 psum tiles x 2 rows)
    """
    assert nb % 2 == 0 and hh % rstrip == 0 and rstrip == 16
    nstrips = hh // rstrip
    ntiles = rstrip // 2  # 8 psum tiles (2 output rows each) per strip

    nc = bacc.Bacc("TRN2", target_bir_lowering=False, debug=False)

    # x arrives pre-padded with the SAME-conv halo: [nb, I, hh+2, ww+2]
    hpad, wpad = hh + 2, ww + 2
    x_t = nc.dram_tensor("x", [nb, I, hpad, wpad], BF16, kind="ExternalInput").ap()
    wl_t = nc.dram_tensor("wL", [O, I * D * D], BF16, kind="ExternalInput").ap()
    wdiag_t = nc.dram_tensor("wdiag", [O, I * D], F32, kind="ExternalInput").ap()
    wloc_t = nc.dram_tensor("wloc", [O, I * D], F32, kind="ExternalInput").ap()
    epsw_t = nc.dram_tensor("epsw", [O, I * D], F32, kind="ExternalInput").ap()
    ident_t = nc.dram_tensor("ident", [O, O], BF16, kind="ExternalInput").ap()
    bias3_t = nc.dram_tensor("bias3", [3, O], BF16, kind="ExternalInput").ap()
    out_t = nc.dram_tensor("out", [nb, O, hh, ww], BF16, kind="ExternalOutput").ap()

    with TileContext(nc) as tc, ExitStack() as stack:
        # ---------------- weight + bias sampling (one-time prologue) --------
        cp = stack.enter_context(tc.tile_pool(name="consts", bufs=1))
        wl = cp.tile([O, I * D * D], BF16, name="wl", tag="wl")
        wdiag = cp.tile([O, I * D], F32, name="wdiag", tag="wdiag")
        wloc = cp.tile([O, I * D], F32, name="wloc_s", tag="wloc_s")
        epsw = cp.tile([O, I * D], F32, name="epsw_s", tag="epsw_s")
        ident = cp.tile([O, O], BF16, name="ident_s", tag="ident_s")
        b3 = cp.tile([O, 3], F32, name="b3", tag="b3")
        sp = cp.tile([O, I * D], F32, name="sp", tag="sp")
        tmp = cp.tile([O, I * D], F32, name="tmp", tag="tmp")
        wsamp = cp.tile([O, I * D], BF16, name="wsamp", tag="wsamp")
        bias = cp.tile([128, 1], F32, name="bias", tag="bias")
        # 9 block-diagonal lhsT tiles, stored side by side: [128, 9*128] bf16
        wts = cp.tile([128, D * 128], BF16, name="wts", tag="wts")
        sp_b = cp.tile([O, 1], F32, name="sp_b", tag="sp_b")

        b3p = cp.tile([3, O], BF16, name="b3p", tag="b3p")
        tmp2 = cp.tile([O, I * D], F32, name="tmp2", tag="tmp2")
        # input DMAs spread over four engine queues so the triggers
        # (~0.7us DIRECT2D each) issue in parallel at t~0
        q = I * D * D // 4
        with tc.high_priority():
            nc.sync.dma_start(wdiag[:], wdiag_t[:])
            for c in range(4):
                nc.sync.dma_start(
                    wl[:, c * q : (c + 1) * q], wl_t[:, c * q : (c + 1) * q]
                )
            nc.scalar.dma_start(wloc[:], wloc_t[:])

        # PE warm-up feed: zero tiles via GpSimd (idle queue, no input deps)
        # so the warm-up matmuls can start within ~1us of kernel entry.
        identr = cp.tile([O, O], F32R, name="identr", tag="identr")
        junk = cp.tile([O, 256], F32R, name="junk", tag="junk")
        with tc.high_priority():
            nc.gpsimd.memset(identr[:].bitcast(F32), 0.0)
            nc.gpsimd.memset(junk[:].bitcast(F32), 0.0)
        nc.gpsimd.memset(wts[:], 0.0)
        with tc.high_priority():
            nc.gpsimd.dma_start(epsw[:], epsw_t[:])
            nc.scalar.dma_start(ident[:], ident_t[:])
            nc.scalar.dma_start(b3p[:], bias3_t[:])

        # PE warm-up: the HAM clock gate needs ~3.4us of sustained matmul
        # activity to lift the PE from 1.2 to 2.4 GHz, and re-throttles after
        # ~3.4us idle. One long accumulation group (no inter-matmul
        # semaphores) bridges the PE from kernel entry to the transposes.
        wp_pool = tc.tile_pool(name="wp", bufs=1, space="PSUM")
        with wp_pool as wp:
            warm = wp.tile([O, 256], F32, name="warm")
            for k in range(n_warm):
                nc.tensor.matmul(
                    warm[:], identr[:], junk[:],
                    start=(k == 0), stop=(k == n_warm - 1),
                )

        # strict-lower products need only wl+epsw, so they run CONCURRENTLY
        # with the softplus ACT chain below, split VectorE (b<4) / GpSimd
        # (b>=4). Each b gets its own scratch region tr[b][o, i, slot=a-1]
        # (slots [b,8) valid) so the 8 products are data-race free and can
        # then be tree-reduced over aligned slot views.
        tr = cp.tile([O, D - 1, I, D - 1], F32, name="tr", tag="tr")

        def tr_view(b, s0):
            # tr[b][:, :, s0:8]
            t = tr[:]
            return bass.AP(
                tensor=t.tensor,
                offset=t.offset + b * I * (D - 1) + s0,
                ap=[list(p) for p in t.ap[:1]]
                + [[D - 1, I], [1, (D - 1) - s0]],
            )

        for b in range(D - 1):
            na = D - 1 - b  # taps strictly below the diagonal
            wl_b = bass.AP(
                tensor=wl[:].tensor,
                offset=wl[:].offset + (b + 1) * D + b,
                ap=[list(p) for p in wl[:].ap[:1]] + [[D * D, I], [D, na]],
            )
            eps_b = bass.AP(
                tensor=epsw[:].tensor,
                offset=epsw[:].offset + b,
                ap=[list(p) for p in epsw[:].ap[:1]] + [[D, I], [0, na]],
            )
            eng = nc.vector if b < 3 else nc.gpsimd
            eng.tensor_tensor(tr_view(b, b), wl_b, eps_b, mybir.AluOpType.mult)

        # softplus of the per-(o,i) diagonals: wl free layout is (i, d=a*9+b);
        # diagonal entries sit at d = 10*a  ->  sp layout (i, a).
        # softplus(x) = ln(exp(x) + 1): there is no Softplus LUT.
        with tc.high_priority():
            nc.scalar.activation(sp[:], wdiag[:], AF.Exp)
            nc.scalar.activation(sp[:], sp[:], AF.Ln, bias=1.0)

        # wsamp = wloc + softplus(diag) * eps  (the b == a term of L @ eps)
        nc.vector.tensor_mul(tmp2[:], sp[:], epsw[:])
        nc.vector.tensor_add(wsamp[:], wloc[:], tmp2[:])

        # tree-reduce the 8 scratch regions over aligned slot views, then
        # fold into wsamp: depth 4, vector and gpsimd in parallel
        for dst, srcb, s0, eng in (
            (0, 1, 1, nc.vector),
            (4, 5, 5, nc.gpsimd),
            (6, 7, 7, nc.gpsimd),
            (0, 2, 2, nc.vector),
            (3, 4, 4, nc.gpsimd),
            (3, 6, 6, nc.gpsimd),
        ):
            eng.tensor_add(tr_view(dst, s0), tr_view(dst, s0), tr_view(srcb, s0))
        nc.vector.tensor_add(tr_view(0, 3), tr_view(0, 3), tr_view(3, 3))
        ws_lower = bass.AP(
            tensor=wsamp[:].tensor,
            offset=wsamp[:].offset + 1,
            ap=[list(p) for p in wsamp[:].ap[:1]] + [[D, I], [1, D - 1]],
        )
        nc.vector.tensor_add(ws_lower, ws_lower, tr_view(0, 0))

        # build the 9 block-diagonal lhsT tiles:
        #   wts[:, a*128:(a+1)*128] = [[T_a, 0], [0, T_a]],  T_a[i,o] = wsamp[o, i*9+a]
        with tc.tile_pool(name="pt", bufs=1, space="PSUM") as ptp:
            # transpose the 9 taps, packed 5 + 4 into two PSUM banks, then
            # two strided copies into the lhsT tile (disjoint column ranges:
            # start=True only on the first write of each bank)
            ptA = ptp.tile([O, 1024], BF16, name="ptA")
            ptB = ptp.tile([O, 1024], BF16, name="ptB")
            for a in range(D):
                w_a = bass.AP(
                    tensor=wsamp[:].tensor,
                    offset=wsamp[:].offset + a,
                    ap=[list(p) for p in wsamp[:].ap[:1]] + [[D, I]],
                )
                dst_pt = ptA if a < 5 else ptB
                c = a if a < 5 else a - 5
                nc.tensor.matmul(
                    dst_pt[:, c * O : (c + 1) * O],
                    w_a,
                    ident[:],
                    is_transpose=True,
                    start=(c == 0),
                    stop=(c == (4 if a < 5 else 3)),
                    skip_group_check=True,
                )
            # bias3 arrives as [3, 64]; transpose to [64, 3] on the PE (a
            # partition-major DMA of 64x3 elements costs ~17us in
            # descriptors). Deferred here: bias only gates the first evac.
            bp_ps = ptp.tile([O, 512], F32, name="bp_ps")
            nc.tensor.matmul(
                bp_ps[:, 0:3], b3p[:], ident[0:3, 0:3], start=True, stop=True
            )
            for pt_t, a0, na_t in ((ptA, 0, 5), (ptB, 5, 4)):
                psrc = bass.AP(
                    tensor=pt_t[:].tensor,
                    offset=pt_t[:].offset,
                    ap=[list(p) for p in pt_t[:].ap[:1]] + [[O, na_t], [1, O]],
                )
                dst = bass.AP(
                    tensor=wts[0:O].tensor,
                    offset=wts[0:O].offset + a0 * 128,
                    ap=[list(p) for p in wts[0:O].ap[:1]] + [[128, na_t], [1, O]],
                )
                nc.vector.tensor_copy(dst, psrc)
            nc.vector.tensor_copy(b3[:], bp_ps[:, 0:3])
        # partition-shifted copy of the diagonal blocks (SBUF -> SBUF)
        for a0, na_t in ((0, 5), (5, 4)):
            srcs = bass.AP(
                tensor=wts[0:O].tensor,
                offset=wts[0:O].offset + a0 * 128,
                ap=[list(p) for p in wts[0:O].ap[:1]] + [[128, na_t], [1, O]],
            )
            dsth = bass.AP(
                tensor=wts[O:128].tensor,
                offset=wts[O:128].offset + a0 * 128 + O,
                ap=[list(p) for p in wts[O:128].ap[:1]] + [[128, na_t], [1, O]],
            )
            nc.scalar.dma_start(dsth, srcs)
        # bias = bias_loc + eps_b * softplus(bias_ro); off the conv critical
        # path (first evac happens ~2us after the first conv matmul)
        nc.scalar.activation(sp_b[:], b3[:, 1:2], AF.Exp)
        nc.scalar.activation(sp_b[:], sp_b[:], AF.Ln, bias=1.0)
        nc.vector.tensor_mul(sp_b[:], sp_b[:], b3[:, 2:3])
        nc.vector.tensor_add(bias[0:O, :], b3[:, 0:1], sp_b[:])
        nc.scalar.dma_start(bias[O:128, :], bias[0:O, :])

        # ---------------- convolution ---------------------------------------
        # xs strip tiles are [128, rstrip+2, wp] slices of the PRE-PADDED
        # input: strip at output-row base h0 loads padded rows [h0, h0+18),
        # one contiguous descriptor per partition. SAME padding is baked into
        # the HBM layout, so there are no halo memsets and no edge cases.
        xp = stack.enter_context(tc.tile_pool(name="xstrip", bufs=x_bufs))
        op = stack.enter_context(tc.tile_pool(name="ostrip", bufs=o_bufs))
        pp = stack.enter_context(tc.tile_pool(name="acc", bufs=1, space="PSUM"))
        for pair in range(nb // 2):
            n0 = 2 * pair
            for s in range(nstrips):
                h0 = s * rstrip
                last = pair == nb // 2 - 1 and s == nstrips - 1

                xs = xp.tile([128, rstrip + 2, wpad], BF16, name="xs")
                src = x_t[n0 : n0 + 2, :, h0 : h0 + rstrip + 2, :].rearrange(
                    "n i h w -> (n i) h w"
                )
                nc.sync.dma_start(xs[:], src)

                os_ = op.tile([128, rstrip, ww], BF16, name="os_")
                # each acc tile is a full 2KB PSUM bank (512 f32); only the
                # first 2*ww elements are used - a matmul output must not
                # straddle a bank boundary
                accs = [
                    pp.tile([128, 512], F32, name=f"acc{j}") for j in range(ntiles)
                ]
                for t, a in enumerate(TAP_ORDER):
                    dy, dx = a // 3 - 1, a % 3 - 1
                    for j in range(ntiles):
                        xs_f = xs[:]
                        rhs = bass.AP(
                            tensor=xs_f.tensor,
                            offset=xs_f.offset + (2 * j + dy + 1) * wpad + (1 + dx),
                            ap=[list(p) for p in xs_f.ap[:1]] + [[wpad, 2], [1, ww]],
                        )
                        nc.tensor.matmul(
                            accs[j][:, 0 : 2 * ww],
                            wts[:, a * 128 : (a + 1) * 128],
                            rhs,
                            start=(t == 0),
                            stop=(t == D - 1),
                            skip_group_check=True,
                        )
                # evacuate the 8 banks, alternating ScalarE / VectorE, with
                # the bias add fused and bf16 downcast on the write
                for j in range(ntiles):
                    dst = os_[:, 2 * j : 2 * j + 2, :]
                    acc_v = accs[j][:, 0 : 2 * ww]
                    if j % 2 == 0:
                        nc.scalar.activation(
                            dst, acc_v, AF.Identity, bias=bias[:, 0:1]
                        )
                    else:
                        nc.vector.tensor_scalar_add(dst, acc_v, bias[:, 0:1])
                d = out_t[n0 : n0 + 2, :, h0 : h0 + rstrip, :].rearrange(
                    "n i h w -> (n i) h w"
                )
                nc.sync.dma_start(d, os_[:])

    nc.compile()
    _dedup_ldweights(nc)
    return nc


def _dedup_ldweights(nc):
    """Drop redundant PE weight (re)loads.

    Legalization splits every InstMatmult into InstLdweights + a non-loading
    InstMatmult. The tap-outer conv order makes 8 consecutive matmuls share
    one lhsT tile, so 7 of the 8 loads reload identical weights; on HW each
    LDWEIGHTS costs ~100ns SERIALIZED with the ~93ns matmul. The inserted
    InstLdweights carry no semaphore updates, so any wait-free reload of the
    currently-resident weights AP can be deleted without changing sync.
    """
    PE = mybir.EngineType.PE
    for fn in nc.m.functions:
        for bl in fn.blocks:
            keep = []
            last_key = None
            changed = False
            for i in bl.instructions:
                if isinstance(i, mybir.InstLdweights):
                    key = str(i.ins[0])
                    if key == last_key and not i.has_wait() and not i.has_update():
                        changed = True
                        continue
                    last_key = key
                elif (
                    isinstance(i, mybir.InstMatmult)
                    and i.ldweights is False
                    and not i.is_transpose
                ):
                    pass  # uses resident weights, does not change them
                elif getattr(i, "engine", None) == PE:
                    last_key = None
                keep.append(i)
            if changed:
                bl.instructions = keep


_CACHED_NC = None


def _host_inputs(x_shard, weight_loc, weight_L, bias_loc, bias_ro, eps_w, eps_b):
    xb = np.asarray(x_shard, np.float32).astype(ml_dtypes.bfloat16)
    xb = np.pad(xb, ((0, 0), (0, 0), (1, 1), (1, 1)))
    wLf = np.asarray(weight_L, np.float32).reshape(O, I, D, D)
    return {
        "x": np.ascontiguousarray(xb),
        "wL": np.ascontiguousarray(
            wLf.reshape(O, I * D * D).astype(ml_dtypes.bfloat16)
        ),
        "wdiag": np.ascontiguousarray(
            np.einsum("oiaa->oia", wLf).reshape(O, I * D)
        ),
        "wloc": np.ascontiguousarray(weight_loc.reshape(O, I * D), np.float32),
        "epsw": np.ascontiguousarray(eps_w.reshape(O, I * D), np.float32),
        "ident": np.eye(O, dtype=np.float32).astype(ml_dtypes.bfloat16),
        "bias3": np.ascontiguousarray(
            np.stack([bias_loc, bias_ro, eps_b])
            .astype(np.float32)
            .astype(ml_dtypes.bfloat16)
        ),
    }


def kernel(x, weight_loc, weight_L, bias_loc, bias_ro, eps_w, eps_b):
    global _CACHED_NC
    from concourse.bass_utils import run_bass_kernel_spmd

    x = np.asarray(x, np.float32)
    nb = x.shape[0] // N_CORES
    if _CACHED_NC is None:
        _CACHED_NC = build_nc(nb=nb)
    nc = _CACHED_NC

    in_maps = [
        _host_inputs(
            x[c * nb : (c + 1) * nb],
            np.asarray(weight_loc),
            np.asarray(weight_L),
            np.asarray(bias_loc),
            np.asarray(bias_ro),
            np.asarray(eps_w),
            np.asarray(eps_b),
        )
        for c in range(N_CORES)
    ]
    res = run_bass_kernel_spmd(nc, in_maps, list(range(N_CORES)))
    return np.concatenate(
        [np.asarray(res.results[c]["out"], np.float32) for c in range(N_CORES)], axis=0
    )


# revision 32
# speedup vs baseline: 1.0032x; 1.0032x over previous
"""Trainium2 Bass kernel for nn_Conv2DExperimental (MVN-sampled 3x3 conv).

Computation (per the nn.Module):
  L    = tril(weight_L, -1) + softplus(diag(weight_L)) * I      # [O,I,D,D], D=9
  w    = weight_loc + einsum('oiab,oib->oia', L, eps_w)         # [O,I,3,3]
  b    = bias_loc + eps_b * softplus(bias_ro)                   # [O]
  out  = conv2d(x, w, SAME, NCHW) + b

Distribution: data-parallel over the batch dim of x (32 images -> 8 cores x 4),
with the weight sampling replicated on every core (it is tiny).

Per-core kernel:
  - sampling runs on VectorE/ScalarE with O on the partition dim; the 9 sampled
    64x64 tap matrices are PE-transposed into block-diagonal [128,128] bf16
    lhsT tiles pairing two images per matmul (partitions = (image, channel)).
  - conv runs tap-OUTER over 8 PSUM banks (16 output rows per strip): the 8
    consecutive matmuls of one tap pass share the same lhsT tile, amortizing
    the per-matmul weight-(re)load/dispatch bubble 8x.
  - x and out live in HBM as bf16 (host casts); SBUF strips are packed
    (no halo columns) - SAME padding is realized by restricting the
    out-column/row ranges of edge-tap matmuls, so every strip load/store is
    one contiguous descriptor per partition.
  - PSUM evacuation (bias add fused, bf16 out) alternates ScalarE/VectorE.
"""

import sys
from contextlib import ExitStack

for _p in ("/opt/trn_rl_repo",):
    if _p not in sys.path:
        sys.path.insert(0, _p)

import numpy as np
import ml_dtypes

import concourse.bass as bass
import concourse.bacc as bacc
import concourse.mybir as mybir
from concourse.tile import TileContext

F32 = mybir.dt.float32
F32R = mybir.dt.float32r
BF16 = mybir.dt.bfloat16
AF = mybir.ActivationFunctionType

N_CORES = 8
O = 64
I = 64
KK = 3
D = KK * KK  # 9

# tap order within each PSUM accumulation group: (0,0) first (it is never
# edge-restricted, so the start=True matmul always zeroes the full bank tile)
TAP_ORDER = [4, 0, 1, 2, 3, 5, 6, 7, 8]


def build_nc(nb=4, hh=224, ww=224, rstrip=16, x_bufs=3, o_bufs=2, n_warm=74):
    """Build the per-core Bass program.

    nb: images per core (must be even: images are processed in pairs)
    hh, ww: spatial dims; rstrip: output rows per strip (8 psum tiles x 2 rows)
    """
    assert nb % 2 == 0 and hh % rstrip == 0 and rstrip == 16
    nstrips = hh // rstrip
    ntiles = rstrip // 2  # 8 psum tiles (2 output rows each) per strip

    nc = bacc.Bacc("TRN2", target_bir_lowering=False, debug=False)

    # x arrives pre-padded with the SAME-conv halo: [nb, I, hh+2, ww+2]
    hpad, wpad = hh + 2, ww + 2
    x_t = nc.dram_tensor("x", [nb, I, hpad, wpad], BF16, kind="ExternalInput").ap()
    wl_t = nc.dram_tensor("wL", [O, I * D * D], BF16, kind="ExternalInput").ap()
    wdiag_t = nc.dram_tensor("wdiag", [O, I * D], F32, kind="ExternalInput").ap()
    wloc_t = nc.dram_tensor("wloc", [O, I * D], F32, kind="ExternalInput").ap()
    epsw_t = nc.dram_tensor("epsw", [O, I * D], F32, kind="ExternalInput").ap()
    ident_t = nc.dram_tensor("ident", [O, O], BF16, kind="ExternalInput").ap()
    bias3_t = nc.dram_tensor("bias3", [3, O], BF16, kind="ExternalInput").ap()
    out_t = nc.dram_tensor("out", [nb, O, hh, ww], BF16, kind="ExternalOutput").ap()

    with TileContext(nc) as tc, ExitStack() as stack:
        # ---------------- weight + bias sampling (one-time prologue) --------
        cp = stack.enter_context(tc.tile_pool(name="consts", bufs=1))
        wl = cp.tile([O, I * D * D], BF16, name="wl", tag="wl")
        wdiag = cp.tile([O, I * D], F32, name="wdiag", tag="wdiag")
        wloc = cp.tile([O, I * D], F32, name="wloc_s", tag="wloc_s")
        epsw = cp.tile([O, I * D], F32, name="epsw_s", tag="epsw_s")
        ident = cp.tile([O, O], BF16, name="ident_s", tag="ident_s")
        b3 = cp.tile([O, 3], F32, name="b3", tag="b3")
        sp = cp.tile([O, I * D], F32, name="sp", tag="sp")
        tmp = cp.tile([O, I * D], F32, name="tmp", tag="tmp")
        wsamp = cp.tile([O, I * D], BF16, name="wsamp", tag="wsamp")
        bias = cp.tile([128, 1], F32, name="bias", tag="bias")
        # 9 block-diagonal lhsT tiles, stored side by side: [128, 9*128] bf16
        wts = cp.tile([128, D * 128], BF16, name="wts", tag="wts")
        sp_b = cp.tile([O, 1], F32, name="sp_b", tag="sp_b")

        b3p = cp.tile([3, O], BF16, name="b3p", tag="b3p")
        tmp2 = cp.tile([O, I * D], F32, name="tmp2", tag="tmp2")
        # input DMAs spread over four engine queues so the triggers
        # (~0.7us DIRECT2D each) issue in parallel at t~0
        q = I * D * D // 4
        with tc.high_priority():
            nc.sync.dma_start(wdiag[:], wdiag_t[:])
            for c in range(4):
                nc.sync.dma_start(
                    wl[:, c * q : (c + 1) * q], wl_t[:, c * q : (c + 1) * q]
                )
            nc.scalar.dma_start(wloc[:], wloc_t[:])

        # PE warm-up feed: zero tiles via GpSimd (idle queue, no input deps)
        # so the warm-up matmuls can start within ~1us of kernel entry.
        identr = cp.tile([O, O], F32R, name="identr", tag="identr")
        junk = cp.tile([O, 256], F32R, name="junk", tag="junk")
        with tc.high_priority():
            nc.gpsimd.memset(identr[:].bitcast(F32), 0.0)
            nc.gpsimd.memset(junk[:].bitcast(F32), 0.0)
        nc.gpsimd.memset(wts[:], 0.0)
        with tc.high_priority():
            nc.gpsimd.dma_start(epsw[:], epsw_t[:])
            nc.scalar.dma_start(ident[:], ident_t[:])
            nc.scalar.dma_start(b3p[:], bias3_t[:])

        # PE warm-up: the HAM clock gate needs ~3.4us of sustained matmul
        # activity to lift the PE from 1.2 to 2.4 GHz, and re-throttles after
        # ~3.4us idle. One long accumulation group (no inter-matmul
        # semaphores) bridges the PE from kernel entry to the transposes.
        wp_pool = tc.tile_pool(name="wp", bufs=1, space="PSUM")
        with wp_pool as wp:
            warm = wp.tile([O, 256], F32, name="warm")
            for k in range(n_warm):
                nc.tensor.matmul(
                    warm[:], identr[:], junk[:],
                    start=(k == 0), stop=(k == n_warm - 1),
                )

        # strict-lower products need only wl+epsw, so they run CONCURRENTLY
        # with the softplus ACT chain below, split VectorE (b<4) / GpSimd
        # (b>=4). Each b gets its own scratch region tr[b][o, i, slot=a-1]
        # (slots [b,8) valid) so the 8 products are data-race free and can
        # then be tree-reduced over aligned slot views.
        tr = cp.tile([O, D - 1, I, D - 1], F32, name="tr", tag="tr")

        def tr_view(b, s0):
            # tr[b][:, :, s0:8]
            t = tr[:]
            return bass.AP(
                tensor=t.tensor,
                offset=t.offset + b * I * (D - 1) + s0,
                ap=[list(p) for p in t.ap[:1]]
                + [[D - 1, I], [1, (D - 1) - s0]],
            )

        for b in range(D - 1):
            na = D - 1 - b  # taps strictly below the diagonal
            wl_b = bass.AP(
                tensor=wl[:].tensor,
                offset=wl[:].offset + (b + 1) * D + b,
                ap=[list(p) for p in wl[:].ap[:1]] + [[D * D, I], [D, na]],
            )
            eps_b = bass.AP(
                tensor=epsw[:].tensor,
                offset=epsw[:].offset + b,
                ap=[list(p) for p in epsw[:].ap[:1]] + [[D, I], [0, na]],
            )
            eng = nc.vector if b < 3 else nc.gpsimd
            eng.tensor_tensor(tr_view(b, b), wl_b, eps_b, mybir.AluOpType.mult)

        # softplus of the per-(o,i) diagonals: wl free layout is (i, d=a*9+b);
        # diagonal entries sit at d = 10*a  ->  sp layout (i, a).
        # softplus(x) = ln(exp(x) + 1): there is no Softplus LUT.
        with tc.high_priority():
            nc.scalar.activation(sp[:], wdiag[:], AF.Exp)
            nc.scalar.activation(sp[:], sp[:], AF.Ln, bias=1.0)

        # wsamp = wloc + softplus(diag) * eps  (the b == a term of L @ eps)
        nc.vector.tensor_mul(tmp2[:], sp[:], epsw[:])
        nc.vector.tensor_add(wsamp[:], wloc[:], tmp2[:])

        # tree-reduce the 8 scratch regions over aligned slot views, then
        # fold into wsamp: depth 4, vector and gpsimd in parallel
        for dst, srcb, s0, eng in (
            (0, 1, 1, nc.vector),
            (4, 5, 5, nc.gpsimd),
            (6, 7, 7, nc.gpsimd),
            (0, 2, 2, nc.vector),
            (3, 4, 4, nc.gpsimd),
            (3, 6, 6, nc.gpsimd),
        ):
            eng.tensor_add(tr_view(dst, s0), tr_view(dst, s0), tr_view(srcb, s0))
        nc.vector.tensor_add(tr_view(0, 3), tr_view(0, 3), tr_view(3, 3))
        ws_lower = bass.AP(
            tensor=wsamp[:].tensor,
            offset=wsamp[:].offset + 1,
            ap=[list(p) for p in wsamp[:].ap[:1]] + [[D, I], [1, D - 1]],
        )
        nc.vector.tensor_add(ws_lower, ws_lower, tr_view(0, 0))

        # build the 9 block-diagonal lhsT tiles:
        #   wts[:, a*128:(a+1)*128] = [[T_a, 0], [0, T_a]],  T_a[i,o] = wsamp[o, i*9+a]
        with tc.tile_pool(name="pt", bufs=1, space="PSUM") as ptp:
            # transpose the 9 taps, packed 5 + 4 into two PSUM banks, then
            # two strided copies into the lhsT tile (disjoint column ranges:
            # start=True only on the first write of each bank)
            ptA = ptp.tile([O, 1024], BF16, name="ptA")
            ptB = ptp.tile([O, 1024], BF16, name="ptB")
            for a in range(D):
                w_a = bass.AP(
                    tensor=wsamp[:].tensor,
                    offset=wsamp[:].offset + a,
                    ap=[list(p) for p in wsamp[:].ap[:1]] + [[D, I]],
                )
                dst_pt = ptA if a < 5 else ptB
                c = a if a < 5 else a - 5
                nc.tensor.matmul(
                    dst_pt[:, c * O : (c + 1) * O],
                    w_a,
                    ident[:],
                    is_transpose=True,
                    start=(c == 0),
                    stop=(c == (4 if a < 5 else 3)),
                    skip_group_check=True,
                )
            # bias3 arrives as [3, 64]; transpose to [64, 3] on the PE (a
            # partition-major DMA of 64x3 elements costs ~17us in
            # descriptors). Deferred here: bias only gates the first evac.
            bp_ps = ptp.tile([O, 512], F32, name="bp_ps")
            nc.tensor.matmul(
                bp_ps[:, 0:3], b3p[:], ident[0:3, 0:3], start=True, stop=True
            )
            for pt_t, a0, na_t in ((ptA, 0, 5), (ptB, 5, 4)):
                psrc = bass.AP(
                    tensor=pt_t[:].tensor,
                    offset=pt_t[:].offset,
                    ap=[list(p) for p in pt_t[:].ap[:1]] + [[O, na_t], [1, O]],
                )
                dst = bass.AP(
                    tensor=wts[0:O].tensor,
                    offset=wts[0:O].offset + a0 * 128,
                    ap=[list(p) for p in wts[0:O].ap[:1]] + [[128, na_t], [1, O]],
                )
                nc.vector.tensor_copy(dst, psrc)
            nc.vector.tensor_copy(b3[:], bp_ps[:, 0:3])
        # partition-shifted copy of the diagonal blocks (SBUF -> SBUF)
        for a0, na_t in ((0, 5), (5, 4)):
            srcs = bass.AP(
                tensor=wts[0:O].tensor,
                offset=wts[0:O].offset + a0 * 128,
                ap=[list(p) for p in wts[0:O].ap[:1]] + [[128, na_t], [1, O]],
            )
            dsth = bass.AP(
                tensor=wts[O:128].tensor,
                offset=wts[O:128].offset + a0 * 128 + O,
                ap=[list(p) for p in wts[O:128].ap[:1]] + [[128, na_t], [1, O]],
            )
            nc.scalar.dma_start(dsth, srcs)
        # bias = bias_loc + eps_b * softplus(bias_ro); off the conv critical
        # path (first evac happens ~2us after the first conv matmul)
        nc.scalar.activation(sp_b[:], b3[:, 1:2], AF.Exp)
        nc.scalar.activation(sp_b[:], sp_b[:], AF.Ln, bias=1.0)
        nc.vector.tensor_mul(sp_b[:], sp_b[:], b3[:, 2:3])
        nc.vector.tensor_add(bias[0:O, :], b3[:, 0:1], sp_b[:])
        nc.scalar.dma_start(bias[O:128, :], bias[0:O, :])

        # ---------------- convolution ---------------------------------------
        # xs strip tiles are [128, rstrip+2, wp] slices of the PRE-PADDED
        # input: strip at output-row base h0 loads padded rows [h0, h0+18),
        # one contiguous descriptor per partition. SAME padding is baked into
        # the HBM layout, so there are no halo memsets and no edge cases.
        xp = stack.enter_context(tc.tile_pool(name="xstrip", bufs=x_bufs))
        op = stack.enter_context(tc.tile_pool(name="ostrip", bufs=o_bufs))
        pp = stack.enter_context(tc.tile_pool(name="acc", bufs=1, space="PSUM"))
        for pair in range(nb // 2):
            n0 = 2 * pair
            for s in range(nstrips):
                h0 = s * rstrip
                last = pair == nb // 2 - 1 and s == nstrips - 1

                xs = xp.tile([128, rstrip + 2, wpad], BF16, name="xs")
                src = x_t[n0 : n0 + 2, :, h0 : h0 + rstrip + 2, :].rearrange(
                    "n i h w -> (n i) h w"
                )
                nc.sync.dma_start(xs[:], src)

                os_ = op.tile([128, rstrip, ww], BF16, name="os_")
                # each acc tile is a full 2KB PSUM bank (512 f32); only the
                # first 2*ww elements are used - a matmul output must not
                # straddle a bank boundary
                accs = [
                    pp.tile([128, 512], F32, name=f"acc{j}") for j in range(ntiles)
                ]
                for t, a in enumerate(TAP_ORDER):
                    dy, dx = a // 3 - 1, a % 3 - 1
                    for j in range(ntiles):
                        xs_f = xs[:]
                        rhs = bass.AP(
                            tensor=xs_f.tensor,
                            offset=xs_f.offset + (2 * j + dy + 1) * wpad + (1 + dx),
                            ap=[list(p) for p in xs_f.ap[:1]] + [[wpad, 2], [1, ww]],
                        )
                        nc.tensor.matmul(
                            accs[j][:, 0 : 2 * ww],
                            wts[:, a * 128 : (a + 1) * 128],
                            rhs,
                            start=(t == 0),
                            stop=(t == D - 1),
                            skip_group_check=True,
                        )
                # evacuate the 8 banks, alternating ScalarE / VectorE, with
                # the bias add fused and bf16 downcast on the write
                for j in range(ntiles):
                    dst = os_[:, 2 * j : 2 * j + 2, :]
                    acc_v = accs[j][:, 0 : 2 * ww]
                    on_scalar = (j % 2 == 1) if last else (j % 2 == 0)
                    if on_scalar:
                        nc.scalar.activation(
                            dst, acc_v, AF.Identity, bias=bias[:, 0:1]
                        )
                    else:
                        nc.vector.tensor_scalar_add(dst, acc_v, bias[:, 0:1])
                    if last and j % 4 == 3:
                        # taper: store each 8-row half as soon as its four
                        # banks are evacuated
                        dtap = out_t[
                            n0 : n0 + 2, :, h0 + 2 * j - 6 : h0 + 2 * j + 2, :
                        ].rearrange("n i h w -> (n i) h w")
                        nc.sync.dma_start(dtap, os_[:, 2 * j - 6 : 2 * j + 2, :])
                if not last:
                    d = out_t[n0 : n0 + 2, :, h0 : h0 + rstrip, :].rearrange(
                        "n i h w -> (n i) h w"
                    )
                    nc.sync.dma_start(d, os_[:])

    nc.compile()
    _dedup_ldweights(nc)
    return nc


def _dedup_ldweights(nc):
    """Drop redundant PE weight (re)loads.

    Legalization splits every InstMatmult into InstLdweights + a non-loading
    InstMatmult. The tap-outer conv order makes 8 consecutive matmuls share
    one lhsT tile, so 7 of the 8 loads reload identical weights; on HW each
    LDWEIGHTS costs ~100ns SERIALIZED with the ~93ns matmul. The inserted
    InstLdweights carry no semaphore updates, so any wait-free reload of the
    currently-resident weights AP can be deleted without changing sync.
    """
    PE = mybir.EngineType.PE
    for fn in nc.m.functions:
        for bl in fn.blocks:
            keep = []
            last_key = None
            changed = False
            for i in bl.instructions:
                if isinstance(i, mybir.InstLdweights):
                    key = str(i.ins[0])
                    if key == last_key and not i.has_wait() and not i.has_update():
                        changed = True
                        continue
                    last_key = key
                elif (
                    isinstance(i, mybir.InstMatmult)
                    and i.ldweights is False
                    and not i.is_transpose
                ):
                    pass  # uses resident weights, does not change them
                elif getattr(i, "engine", None) == PE:
                    last_key = None
                keep.append(i)
            if changed:
                bl.instructions = keep


_CACHED_NC = None


def _host_inputs(x_shard, weight_loc, weight_L, bias_loc, bias_ro, eps_w, eps_b):
    xb = np.asarray(x_shard, np.float32).astype(ml_dtypes.bfloat16)
    xb = np.pad(xb, ((0, 0), (0, 0), (1, 1), (1, 1)))
    wLf = np.asarray(weight_L, np.float32).reshape(O, I, D, D)
    return {
        "x": np.ascontiguousarray(xb),
        "wL": np.ascontiguousarray(
            wLf.reshape(O, I * D * D).astype(ml_dtypes.bfloat16)
        ),
        "wdiag": np.ascontiguousarray(
            np.einsum("oiaa->oia", wLf).reshape(O, I * D)
        ),
        "wloc": np.ascontiguousarray(weight_loc.reshape(O, I * D), np.float32),
        "epsw": np.ascontiguousarray(eps_w.reshape(O, I * D), np.float32),
        "ident": np.eye(O, dtype=np.float32).astype(ml_dtypes.bfloat16),
        "bias3": np.ascontiguousarray(
            np.stack([bias_loc, bias_ro, eps_b])
            .astype(np.float32)
            .astype(ml_dtypes.bfloat16)
        ),
    }


def kernel(x, weight_loc, weight_L, bias_loc, bias_ro, eps_w, eps_b):
    global _CACHED_NC
    from concourse.bass_utils import run_bass_kernel_spmd

    x = np.asarray(x, np.float32)
    nb = x.shape[0] // N_CORES
    if _CACHED_NC is None:
        _CACHED_NC = build_nc(nb=nb)
    nc = _CACHED_NC

    in_maps = [
        _host_inputs(
            x[c * nb : (c + 1) * nb],
            np.asarray(weight_loc),
            np.asarray(weight_L),
            np.asarray(bias_loc),
            np.asarray(bias_ro),
            np.asarray(eps_w),
            np.asarray(eps_b),
        )
        for c in range(N_CORES)
    ]
    res = run_bass_kernel_spmd(nc, in_maps, list(range(N_CORES)))
    return np.concatenate(
        [np.asarray(res.results[c]["out"], np.float32) for c in range(N_CORES)], axis=0
    )
